# revision 13
# baseline (speedup 1.0000x reference)
"""PCEN kernel for Trainium2, SPMD across 8 NeuronCores — TensorE EMA.

The EMA  m_t = 0.5 m_{t-1} + 0.5 x_t  has kernel weights 0.5^{j+1} that
underflow to zero (even in bf16) within 128 taps, so per 128-col tile k:

    M[:, k] = W0.T @ x[:, k] + W1.T @ x[:, k-1]     (exact)

with Toeplitz weights W0[tau, t'] = 0.5^{t'-tau+1} (t' >= tau) and
W1[tau, t'] = 0.5^{t'+129-tau} — all exact powers of two in bf16.  This
moves the serial scan (2.02 cyc/col on DVE, 34.5us/core) onto the idle
TensorE in [t, f]-tiled layout, which the host prepares/undoes.

Per-core layout: x_host[p, 128*k + j] = x[f0+j, t = 128*k + p] (bf16).
Pipeline per 16-tile PSUM wave: DMA in -> 16x W0-matmul + 15x W1-matmul
(PSUM fp32) -> ACT Ln in-place in PSUM -> ACT Exp PSUM->SBUF bf16 ->
DVE mul u = x*g (bf16 TT 2x).  Then one ACT table switch and the Sqrt
pass -> DVE sub (fp16 4x) -> DMA out fp16; host upconverts and undoes
the tiling.

ACT timeline (the wall): Ln+Exp 2x(16384+10*172)/1.2 = 30.2us + switch
2.7us + Sqrt 14.8us; everything else hides under it.
"""

from contextlib import ExitStack

import numpy as np

import concourse.tile as tile
from concourse import bacc, mybir
from concourse.bass_utils import run_bass_kernel_spmd

F_FULL = 1024
F_SHARD = 128
T = 16384
N_CORES = 8
EPS = 1e-6
TK = 128  # matmul tile columns
NT = T // TK  # 128 tiles

# tiles per PSUM wave (wave cols = 16*128 = 2048 max -> one half of PSUM,
# bufs=2 fills the 16KB).  Small first wave so ACT starts early, small
# last wave so the tail ln/exp handoff is short.
WAVES = [2, 4, 8, 16, 16, 16, 16, 16, 16, 14, 4]
assert sum(WAVES) == NT

# phase B chunks: wide (ACT-bound), small tail so the drain is short.
CHUNKS_B = [8192, 4096, 2048, 1024, 512, 512]
CHUNKS_MUL = [4096, 4096, 4096, 4096]
assert sum(CHUNKS_B) == T and sum(CHUNKS_MUL) == T

_cache: dict = {}


def build(alpha: float, r: float, delta: float):
    assert abs(r - 0.5) < 1e-6, "kernel hardcodes r=0.5 (sqrt epilogue)"
    delta_r = float(np.float32(delta) ** np.float32(r))

    nc = bacc.Bacc(
        "TRN2", target_bir_lowering=False, debug=False, num_devices=N_CORES
    )
    f32 = mybir.dt.float32
    bf16 = mybir.dt.bfloat16
    fp16 = mybir.dt.float16
    x_d = nc.dram_tensor("data", [F_SHARD, T], bf16, kind="ExternalInput").ap()
    w_d = nc.dram_tensor("w", [TK, 2 * TK], bf16, kind="ExternalInput").ap()
    o_d = nc.dram_tensor("out", [F_SHARD, T], fp16, kind="ExternalOutput").ap()

    with tile.TileContext(nc) as tc, ExitStack() as ctx:
        constp = ctx.enter_context(tc.tile_pool(name="const", bufs=1))
        xfullp = ctx.enter_context(tc.tile_pool(name="xfull", bufs=1))
        gfullp = ctx.enter_context(tc.tile_pool(name="gfull", bufs=1))
        sfullp = ctx.enter_context(tc.tile_pool(name="sfull", bufs=1))
        psp = ctx.enter_context(
            tc.tile_pool(name="ps", bufs=2, space="PSUM")
        )

        wt = constp.tile([TK, 2 * TK], bf16, tag="w")
        nc.sync.dma_start(wt[:], w_d[:])
        eps_b = constp.tile([F_SHARD, 1], f32, tag="epsb")
        nc.vector.memset(eps_b[:], EPS)
        delta_b = constp.tile([F_SHARD, 1], f32, tag="deltab")
        nc.vector.memset(delta_b[:], float(delta))

        x_full = xfullp.tile([F_SHARD, T], bf16)
        g_full = gfullp.tile([F_SHARD, T], bf16)
        s_full = sfullp.tile([F_SHARD, T], fp16)

        LNEXP_SET = 6  # natural_log_exp_and_others
        nc.scalar.add_instruction(
            mybir.InstLoadActFuncSet(
                name=nc.get_next_instruction_name(),
                act_func_set_id=LNEXP_SET,
                ins=[],
                outs=[],
            )
        )

        last_exp = None
        ln_insts = []
        gk = 0  # global tile index
        for w, ntiles in enumerate(WAVES):
            cols = ntiles * TK
            wsl = slice(gk * TK, gk * TK + cols)
            nc.sync.dma_start(x_full[:, wsl], x_d[:, wsl])
            ps = psp.tile([F_SHARD, 16 * TK], f32)
            # W0 and W1 matmuls must be ADJACENT per tile: PSUM
            # accumulation groups are consecutive instructions — with
            # W0s grouped first, only the last tile per bank survived.
            for i in range(ntiles):
                k = gk + i
                osl = slice(i * TK, (i + 1) * TK)
                nc.tensor.matmul(
                    ps[:, osl],
                    wt[:, :TK],
                    x_full[:, k * TK : (k + 1) * TK],
                    start=True,
                    stop=(k == 0),
                )
                if k > 0:
                    nc.tensor.matmul(
                        ps[:, osl],
                        wt[:, TK:],
                        x_full[:, (k - 1) * TK : k * TK],
                        start=False,
                        stop=True,
                    )
            ln_i = nc.scalar.activation(
                ps[:, :cols],
                ps[:, :cols],
                mybir.ActivationFunctionType.Ln,
                bias=eps_b[:],
            )
            ln_insts.append(ln_i)
            if last_exp is not None:
                # pin ACT order Ln0,Exp0,Ln1,Exp1,...
                tile.add_dep_helper(
                    ln_i.ins, last_exp.ins, sync=False, reason="act ln/exp order"
                )
            last_exp = nc.scalar.activation(
                g_full[:, wsl],
                ps[:, :cols],
                mybir.ActivationFunctionType.Exp,
                scale=-float(alpha),
            )
            gk += ntiles

        # muls can start as soon as the Exps covering their span are done
        # (DVE is otherwise idle in phase A)
        pos = 0
        for c in CHUNKS_MUL:
            sl = slice(pos, pos + c)
            nc.vector.tensor_mul(g_full[:, sl], x_full[:, sl], g_full[:, sl])
            pos += c

        pos = 0
        for c in CHUNKS_B:
            sl = slice(pos, pos + c)
            s = nc.scalar.activation(
                s_full[:, sl],
                g_full[:, sl],
                mybir.ActivationFunctionType.Sqrt,
                bias=delta_b[:],
            )
            # keep every sqrt after the last exp on ACT: one table switch
            tile.add_dep_helper(
                s.ins, last_exp.ins, sync=False, reason="act table phase order"
            )
            nc.vector.tensor_scalar_sub(s_full[:, sl], s_full[:, sl], delta_r)
            nc.sync.dma_start(o_d[:, sl], s_full[:, sl])
            pos += c

    nc.compile()
    return nc


def _get_nc(alpha: float, r: float, delta: float):
    key = (alpha, r, delta)
    if key not in _cache:
        _cache[key] = build(alpha, r, delta)
    return _cache[key]


def _weights():
    import ml_dtypes

    j = np.arange(TK)
    tp, tau = np.meshgrid(j, j, indexing="ij")  # [t', tau]
    # W[tau, t'] layouts: exponent arrays built as [t', tau] then transposed
    e0 = tp - tau + 1.0
    w0 = np.where(tp >= tau, np.float32(2.0) ** (-e0), 0.0).astype(np.float32)
    e1 = tp + 129.0 - tau
    w1 = (np.float32(2.0) ** (-e1)).astype(np.float32)
    return (
        np.ascontiguousarray(w0.T).astype(ml_dtypes.bfloat16),
        np.ascontiguousarray(w1.T).astype(ml_dtypes.bfloat16),
    )


def make_in_maps(data: np.ndarray):
    import ml_dtypes

    x = np.asarray(data)[0].astype(ml_dtypes.bfloat16)  # [1024, 16384]
    w0, w1 = _weights()
    w01 = np.ascontiguousarray(np.concatenate([w0, w1], axis=1))
    maps = []
    for k in range(N_CORES):
        xc = x[k * F_SHARD : (k + 1) * F_SHARD]  # [128 f, 16384 t]
        # [f, tile, p] -> [p, tile, f] so SBUF partition p holds time
        # step 128*tile + p and free dim runs over (tile, f)
        xt = np.ascontiguousarray(
            xc.reshape(F_SHARD, NT, TK).transpose(2, 1, 0)
        ).reshape(F_SHARD, T)
        maps.append({"data": xt, "w": w01})
    return maps


def kernel(data, alpha, r, delta):
    a = float(np.asarray(alpha))
    rr = float(np.asarray(r))
    d = float(np.asarray(delta))
    nc = _get_nc(a, rr, d)
    in_maps = make_in_maps(data)
    res = run_bass_kernel_spmd(nc, in_maps, core_ids=list(range(N_CORES))).results
    parts = []
    for k in range(N_CORES):
        s = np.asarray(res[k]["out"]).astype(np.float32)  # [p, (tile, f)]
        parts.append(
            s.reshape(F_SHARD, NT, F_SHARD).transpose(2, 1, 0).reshape(F_SHARD, T)
        )
    return np.concatenate(parts, axis=0)[None]


# revision 14
# speedup vs baseline: 1.0119x; 1.0119x over previous
"""PCEN kernel for Trainium2, SPMD across 8 NeuronCores — TensorE EMA.

The EMA  m_t = 0.5 m_{t-1} + 0.5 x_t  has kernel weights 0.5^{j+1} that
underflow to zero (even in bf16) within 128 taps, so per 128-col tile k:

    M[:, k] = W0.T @ x[:, k] + W1.T @ x[:, k-1]     (exact)

with Toeplitz weights W0[tau, t'] = 0.5^{t'-tau+1} (t' >= tau) and
W1[tau, t'] = 0.5^{t'+129-tau} — all exact powers of two in bf16.  This
moves the serial scan (2.02 cyc/col on DVE, 34.5us/core) onto the idle
TensorE in [t, f]-tiled layout, which the host prepares/undoes.

Per-core layout: x_host[p, 128*k + j] = x[f0+j, t = 128*k + p] (bf16).
Pipeline per 16-tile PSUM wave: DMA in -> 16x W0-matmul + 15x W1-matmul
(PSUM fp32) -> ACT Ln in-place in PSUM -> ACT Exp PSUM->SBUF bf16 ->
DVE mul u = x*g (bf16 TT 2x).  Then one ACT table switch and the Sqrt
pass -> DVE sub (fp16 4x) -> DMA out fp16; host upconverts and undoes
the tiling.

ACT timeline (the wall): Ln+Exp 2x(16384+10*172)/1.2 = 30.2us + switch
2.7us + Sqrt 14.8us; everything else hides under it.
"""

from contextlib import ExitStack

import numpy as np

import concourse.tile as tile
from concourse import bacc, mybir
from concourse.bass_utils import run_bass_kernel_spmd

F_FULL = 1024
F_SHARD = 128
T = 16384
N_CORES = 8
EPS = 1e-6
TK = 128  # matmul tile columns
NT = T // TK  # 128 tiles

# tiles per PSUM wave (wave cols = 16*128 = 2048 max -> one half of PSUM,
# bufs=2 fills the 16KB).  Small first wave so ACT starts early, small
# last wave so the tail ln/exp handoff is short.
WAVES = [2, 4, 8, 16, 16, 16, 16, 16, 16, 14, 4]
assert sum(WAVES) == NT

# phase B chunks: wide (ACT-bound), small tail so the drain is short.
CHUNKS_B = [8192, 4096, 2048, 1024, 512, 512]
CHUNKS_MUL = [4096, 4096, 4096, 4096]
assert sum(CHUNKS_B) == T and sum(CHUNKS_MUL) == T

_cache: dict = {}


def build(alpha: float, r: float, delta: float):
    assert abs(r - 0.5) < 1e-6, "kernel hardcodes r=0.5 (sqrt epilogue)"
    delta_r = float(np.float32(delta) ** np.float32(r))

    nc = bacc.Bacc(
        "TRN2", target_bir_lowering=False, debug=False, num_devices=N_CORES
    )
    f32 = mybir.dt.float32
    bf16 = mybir.dt.bfloat16
    fp16 = mybir.dt.float16
    x_d = nc.dram_tensor("data", [F_SHARD, T], bf16, kind="ExternalInput").ap()
    w_d = nc.dram_tensor("w", [TK, 2 * TK], bf16, kind="ExternalInput").ap()
    o_d = nc.dram_tensor("out", [F_SHARD, T], fp16, kind="ExternalOutput").ap()

    with tile.TileContext(nc) as tc, ExitStack() as ctx:
        constp = ctx.enter_context(tc.tile_pool(name="const", bufs=1))
        xfullp = ctx.enter_context(tc.tile_pool(name="xfull", bufs=1))
        gfullp = ctx.enter_context(tc.tile_pool(name="gfull", bufs=1))
        sfullp = ctx.enter_context(tc.tile_pool(name="sfull", bufs=1))
        psp = ctx.enter_context(
            tc.tile_pool(name="ps", bufs=2, space="PSUM")
        )

        wt = constp.tile([TK, 2 * TK], bf16, tag="w")
        nc.sync.dma_start(wt[:], w_d[:])
        eps_b = constp.tile([F_SHARD, 1], f32, tag="epsb")
        nc.vector.memset(eps_b[:], EPS)
        delta_b = constp.tile([F_SHARD, 1], f32, tag="deltab")
        nc.vector.memset(delta_b[:], float(delta))

        x_full = xfullp.tile([F_SHARD, T], bf16)
        g_full = gfullp.tile([F_SHARD, T], bf16)
        s_full = sfullp.tile([F_SHARD, T], fp16)

        LNEXP_SET = 6  # natural_log_exp_and_others
        nc.scalar.add_instruction(
            mybir.InstLoadActFuncSet(
                name=nc.get_next_instruction_name(),
                act_func_set_id=LNEXP_SET,
                ins=[],
                outs=[],
            )
        )

        last_exp = None
        ln_insts = []
        gk = 0  # global tile index
        for w, ntiles in enumerate(WAVES):
            cols = ntiles * TK
            wsl = slice(gk * TK, gk * TK + cols)
            nc.sync.dma_start(x_full[:, wsl], x_d[:, wsl])
            ps = psp.tile([F_SHARD, 16 * TK], f32)
            # W0 and W1 matmuls must be ADJACENT per output region: PSUM
            # accumulation groups are consecutive instructions — with
            # W0s grouped first, only the last tile per bank survived.
            # Pair two tiles per matmul (rhs N=2*TK) to halve TensorE
            # instruction count; W1's rhs is the same span shifted one
            # tile left.
            i = 0
            while i < ntiles:
                k = gk + i
                npair = 2 if (i + 1 < ntiles and k > 0) else 1
                osl = slice(i * TK, (i + npair) * TK)
                nc.tensor.matmul(
                    ps[:, osl],
                    wt[:, :TK],
                    x_full[:, k * TK : (k + npair) * TK],
                    start=True,
                    stop=(k == 0),
                )
                if k > 0:
                    nc.tensor.matmul(
                        ps[:, osl],
                        wt[:, TK:],
                        x_full[:, (k - 1) * TK : (k - 1 + npair) * TK],
                        start=False,
                        stop=True,
                    )
                i += npair
            ln_i = nc.scalar.activation(
                ps[:, :cols],
                ps[:, :cols],
                mybir.ActivationFunctionType.Ln,
                bias=eps_b[:],
            )
            ln_insts.append(ln_i)
            if last_exp is not None:
                # pin ACT order Ln0,Exp0,Ln1,Exp1,...
                tile.add_dep_helper(
                    ln_i.ins, last_exp.ins, sync=False, reason="act ln/exp order"
                )
            last_exp = nc.scalar.activation(
                g_full[:, wsl],
                ps[:, :cols],
                mybir.ActivationFunctionType.Exp,
                scale=-float(alpha),
            )
            gk += ntiles

        # muls can start as soon as the Exps covering their span are done
        # (DVE is otherwise idle in phase A)
        pos = 0
        for c in CHUNKS_MUL:
            sl = slice(pos, pos + c)
            nc.vector.tensor_mul(g_full[:, sl], x_full[:, sl], g_full[:, sl])
            pos += c

        pos = 0
        for c in CHUNKS_B:
            sl = slice(pos, pos + c)
            s = nc.scalar.activation(
                s_full[:, sl],
                g_full[:, sl],
                mybir.ActivationFunctionType.Sqrt,
                bias=delta_b[:],
            )
            # keep every sqrt after the last exp on ACT: one table switch
            tile.add_dep_helper(
                s.ins, last_exp.ins, sync=False, reason="act table phase order"
            )
            nc.vector.tensor_scalar_sub(s_full[:, sl], s_full[:, sl], delta_r)
            nc.sync.dma_start(o_d[:, sl], s_full[:, sl])
            pos += c

    nc.compile()
    return nc


def _get_nc(alpha: float, r: float, delta: float):
    key = (alpha, r, delta)
    if key not in _cache:
        _cache[key] = build(alpha, r, delta)
    return _cache[key]


def _weights():
    import ml_dtypes

    j = np.arange(TK)
    tp, tau = np.meshgrid(j, j, indexing="ij")  # [t', tau]
    # W[tau, t'] layouts: exponent arrays built as [t', tau] then transposed
    e0 = tp - tau + 1.0
    w0 = np.where(tp >= tau, np.float32(2.0) ** (-e0), 0.0).astype(np.float32)
    e1 = tp + 129.0 - tau
    w1 = (np.float32(2.0) ** (-e1)).astype(np.float32)
    return (
        np.ascontiguousarray(w0.T).astype(ml_dtypes.bfloat16),
        np.ascontiguousarray(w1.T).astype(ml_dtypes.bfloat16),
    )


def make_in_maps(data: np.ndarray):
    import ml_dtypes

    x = np.asarray(data)[0].astype(ml_dtypes.bfloat16)  # [1024, 16384]
    w0, w1 = _weights()
    w01 = np.ascontiguousarray(np.concatenate([w0, w1], axis=1))
    maps = []
    for k in range(N_CORES):
        xc = x[k * F_SHARD : (k + 1) * F_SHARD]  # [128 f, 16384 t]
        # [f, tile, p] -> [p, tile, f] so SBUF partition p holds time
        # step 128*tile + p and free dim runs over (tile, f)
        xt = np.ascontiguousarray(
            xc.reshape(F_SHARD, NT, TK).transpose(2, 1, 0)
        ).reshape(F_SHARD, T)
        maps.append({"data": xt, "w": w01})
    return maps


def kernel(data, alpha, r, delta):
    a = float(np.asarray(alpha))
    rr = float(np.asarray(r))
    d = float(np.asarray(delta))
    nc = _get_nc(a, rr, d)
    in_maps = make_in_maps(data)
    res = run_bass_kernel_spmd(nc, in_maps, core_ids=list(range(N_CORES))).results
    parts = []
    for k in range(N_CORES):
        s = np.asarray(res[k]["out"]).astype(np.float32)  # [p, (tile, f)]
        parts.append(
            s.reshape(F_SHARD, NT, F_SHARD).transpose(2, 1, 0).reshape(F_SHARD, T)
        )
    return np.concatenate(parts, axis=0)[None]
